# revision 9
# baseline (speedup 1.0000x reference)
"""Trainium2 Bass kernel for nn_Conv2d_20590073217670.

Conv2d: input [32,64,64,64] (NCHW), weight [576,128] (unfold layout:
row = ci*9 + a*3 + b for tap (a,b)), bias [1,128,1,1], stride 1, pad 1.
Output [32,128,64,64].

Strategy: data-parallel over batch — 4 images per NeuronCore, 8 cores.

Per image, implicit GEMM in bf16 (PSUM accumulates fp32; rel err ~2e-3
vs the 2e-2 gate).  The host pre-builds a [128, 66, 66] bf16 tile per
image with zero borders: partitions 0:64 hold the padded image P
(P[r] = img row r-1), partitions 64:128 hold the one-row-down copy U
(U[r] = img row r).  One contiguous HBM DMA per image, zero on-device
data prep.

Every 8-row output block is then uniform (no border special cases):
  out[co,y,x] = sum_{a,b,ci} W[ci,a,b,co] * img[ci, y+a-1, x+b-1]
  - taps (0,b)+(1,b): K=128 matmul, lhsT = [W[:,0,b,:]; W[:,1,b,:]],
    rhs = xim[:, y0:y0+8, b:b+64]           (3 matmuls)
  - tap (2,b): K=128 matmul with ZERO weights on partitions 0:64 and
    W[:,2,b,:] on 64:128, rhs = xim[:, y0+1:y0+9, b:b+64]  (3 matmuls)
    (K=64 matmuls would leave half the PE array idle, which keeps the
    HAM activity monitor from un-throttling the PE clock to 2.4 GHz —
    measured: any K=64 in the stream pins issue at 1.2 GHz.)

~26 dummy matmuls on a scratch SBUF tile run while the first image's
DMA is in flight, so the PE is already warm (K=8/8) when real work
starts.  PSUM eviction (fused bias add) alternates between ScalarE
(activation Identity + bias) and VectorE (tensor_scalar_add) so
neither engine gates the PE.  Output DMAs go in 32-row halves, issued
from ScalarE's hardware DGE so the Sync engine's serialized issue
stream only carries input DMAs.  (Output issues moved back to the
Sync DGE: scalar-issued stores left a straggler packet on a slow
software queue that cost ~5us at the tail.)
"""
import sys

for _p in ("/opt/trn_rl_repo", "/root/.axon_site/_ro/trn_rl_repo"):
    if _p not in sys.path:
        sys.path.append(_p)

import numpy as np
import ml_dtypes
from contextlib import ExitStack

import concourse.bacc as bacc
import concourse.tile as tile
from concourse import mybir
from concourse.bass_utils import run_bass_kernel_spmd

f32 = mybir.dt.float32
bf16 = mybir.dt.bfloat16

N_CORES = 8
NB = 4  # images per core
N_WARM = 10  # dummy matmuls to warm the PE during the input DMA window


def build_nc():
    nc = bacc.Bacc()
    # host-built per-image tile: [P(66x66, zero borders) ; U(P shifted
    # one row up, row 65 = 0)], bf16
    xp = nc.declare_dram_parameter("x", [NB, 128, 66, 66], bf16, isOutput=False)
    # host-packed weights: [128, 6, 128] bf16
    #   w[:, b, :]   = [W[:,0,b,:]; W[:,1,b,:]]  (tap pair a=0,1)
    #   w[:, 3+b, :] = [0         ; W[:,2,b,:]]  (tap 2, zero-padded K=128)
    w = nc.declare_dram_parameter("w", [128, 6, 128], bf16, isOutput=False)
    bias = nc.declare_dram_parameter("b", [128, 1], f32, isOutput=False)
    out = nc.declare_dram_parameter("out", [NB, 128, 64, 64], f32, isOutput=True)

    with tile.TileContext(nc) as tc, ExitStack() as ctx:
        const = ctx.enter_context(tc.tile_pool(name="const", bufs=1))
        xim_pool = ctx.enter_context(tc.tile_pool(name="xim", bufs=3))
        ob_pool = ctx.enter_context(tc.tile_pool(name="ob", bufs=2))
        ps_pool = ctx.enter_context(tc.tile_pool(name="ps", bufs=7, space="PSUM"))
        pw_pool = ctx.enter_context(tc.tile_pool(name="pw", bufs=1, space="PSUM"))

        wt = const.tile([128, 6, 128], bf16)
        bt = const.tile([128, 1], f32)
        dummy = const.tile([128, 640], bf16)

        xims = []
        for n in range(NB):
            xim = xim_pool.tile([128, 66, 66], bf16)
            if n == 0:
                # priority prefix: rows 0:20 cover blocks 0-1, then the
                # weights, so real matmuls start ~4us before the full
                # first image lands
                nc.sync.dma_start(out=xim[:, 0:20, :], in_=xp[n, :, 0:20, :])
                nc.sync.dma_start(out=wt[:], in_=w[:])
                nc.sync.dma_start(out=bt[:], in_=bias[:])
                nc.sync.dma_start(out=xim[:, 20:66, :], in_=xp[n, :, 20:66, :])
            else:
                nc.sync.dma_start(out=xim[:, :, :], in_=xp[n])
            xims.append(xim)

        # PE warm-up: independent matmuls into a scratch PSUM bank while
        # the input DMAs stream.  ~3.4us of sustained PE activity flips
        # the HAM clock gate to 2.4 GHz before the real stream begins.
        nc.gpsimd.memset(dummy[:], 0.0)
        Pw = pw_pool.tile([128, 8, 64], f32)
        for i in range(N_WARM):
            nc.tensor.matmul(
                Pw[:, :, :], dummy[:, 0:128],
                dummy[:, 128:640].rearrange("p (a b) -> p a b", a=8),
                start=True, stop=True,
            )

        for n in range(NB):
            xim = xims[n]
            osb = ob_pool.tile([128, 64, 64], f32)
            for blk in range(8):
                y0 = blk * 8
                P = ps_pool.tile([128, 8, 64], f32)
                for k, b in enumerate((0, 1, 2)):
                    nc.tensor.matmul(
                        P[:, :, :],
                        wt[:, b, :],
                        xim[:, y0:y0 + 8, b:b + 64],
                        start=(k == 0), stop=False,
                    )
                for k, b in enumerate((0, 1, 2)):
                    nc.tensor.matmul(
                        P[:, :, :],
                        wt[:, 3 + b, :],
                        xim[:, y0 + 1:y0 + 9, b:b + 64],
                        start=False, stop=(k == 2),
                    )
                # PSUM evict + bias, alternating engines per bank
                if blk % 2 == 0:
                    nc.scalar.activation(
                        osb[:, y0:y0 + 8, :], P[:, :, :],
                        mybir.ActivationFunctionType.Identity,
                        bias=bt[:, :],
                    )
                else:
                    nc.vector.tensor_scalar_add(
                        osb[:, y0:y0 + 8, :], P[:, :, :], bt[:, :])
                # drain the output in 16-row chunks so the final store
                # after the last matmul is tiny; issue from ScalarE's DGE
                if blk % 2 == 1:
                    r0 = (blk - 1) * 8
                    eng = nc.scalar if n < NB - 1 else nc.sync
                    eng.dma_start(out=out[n, :, r0:r0 + 16, :],
                                  in_=osb[:, r0:r0 + 16, :])

    nc.finalize()
    return nc


_NC = None


def _get_nc():
    global _NC
    if _NC is None:
        _NC = build_nc()
    return _NC


def _prep(inputs):
    x = np.asarray(inputs["input"], dtype=np.float32)
    w = np.asarray(inputs["weight"], dtype=np.float32)
    b = np.asarray(inputs["bias"], dtype=np.float32).reshape(128, 1)

    nimg = x.shape[0]
    xb = x.astype(ml_dtypes.bfloat16)
    xf = np.zeros((nimg, 128, 66, 66), dtype=ml_dtypes.bfloat16)
    xf[:, 0:64, 1:65, 1:65] = xb          # P: rows 1..64 = img rows 0..63
    xf[:, 64:128, 0:64, 1:65] = xb        # U: rows 0..63 = img rows 0..63

    wr = w.reshape(64, 3, 3, 128)  # [ci, a, b, co]
    wa = np.zeros((128, 6, 128), dtype=ml_dtypes.bfloat16)
    for bb in range(3):
        wa[0:64, bb, :] = wr[:, 0, bb, :]
        wa[64:128, bb, :] = wr[:, 1, bb, :]
        wa[64:128, 3 + bb, :] = wr[:, 2, bb, :]
    return xf, wa, np.ascontiguousarray(b)


def kernel(**inputs) -> np.ndarray:
    xf, wa, b = _prep(inputs)
    nc = _get_nc()
    in_maps = [
        {"x": xf[c * NB:(c + 1) * NB], "w": wa, "b": b} for c in range(N_CORES)
    ]
    res = run_bass_kernel_spmd(nc, in_maps, list(range(N_CORES)))
    return np.concatenate([r["out"] for r in res.results], axis=0)


# revision 10
# speedup vs baseline: 1.0166x; 1.0166x over previous
"""Trainium2 Bass kernel for nn_Conv2d_20590073217670.

Conv2d: input [32,64,64,64] (NCHW), weight [576,128] (unfold layout:
row = ci*9 + a*3 + b for tap (a,b)), bias [1,128,1,1], stride 1, pad 1.
Output [32,128,64,64].

Strategy: data-parallel over batch — 4 images per NeuronCore, 8 cores.

Per image, implicit GEMM in bf16 (PSUM accumulates fp32; rel err ~2e-3
vs the 2e-2 gate).  The host pre-builds a [128, 66, 66] bf16 tile per
image with zero borders: partitions 0:64 hold the padded image P
(P[r] = img row r-1), partitions 64:128 hold the one-row-down copy U
(U[r] = img row r).  One contiguous HBM DMA per image, zero on-device
data prep.

Every 8-row output block is then uniform (no border special cases):
  out[co,y,x] = sum_{a,b,ci} W[ci,a,b,co] * img[ci, y+a-1, x+b-1]
  - taps (0,b)+(1,b): K=128 matmul, lhsT = [W[:,0,b,:]; W[:,1,b,:]],
    rhs = xim[:, y0:y0+8, b:b+64]           (3 matmuls)
  - tap (2,b): K=128 matmul with ZERO weights on partitions 0:64 and
    W[:,2,b,:] on 64:128, rhs = xim[:, y0+1:y0+9, b:b+64]  (3 matmuls)
    (K=64 matmuls would leave half the PE array idle, which keeps the
    HAM activity monitor from un-throttling the PE clock to 2.4 GHz —
    measured: any K=64 in the stream pins issue at 1.2 GHz.)

~26 dummy matmuls on a scratch SBUF tile run while the first image's
DMA is in flight, so the PE is already warm (K=8/8) when real work
starts.  PSUM eviction (fused bias add) alternates between ScalarE
(activation Identity + bias) and VectorE (tensor_scalar_add) so
neither engine gates the PE.  Output DMAs go in 32-row halves, issued
from ScalarE's hardware DGE so the Sync engine's serialized issue
stream only carries input DMAs... reverted: ScalarE-issued DMAs
always defer one 16KB packet to a slow queue that flushes only at
kernel end (+5us), so ALL DMAs issue from the Sync DGE.
"""
import sys

for _p in ("/opt/trn_rl_repo", "/root/.axon_site/_ro/trn_rl_repo"):
    if _p not in sys.path:
        sys.path.append(_p)

import numpy as np
import ml_dtypes
from contextlib import ExitStack

import concourse.bacc as bacc
import concourse.tile as tile
from concourse import mybir
from concourse.bass_utils import run_bass_kernel_spmd

f32 = mybir.dt.float32
bf16 = mybir.dt.bfloat16

N_CORES = 8
NB = 4  # images per core
N_WARM = 10  # dummy matmuls to warm the PE during the input DMA window


def build_nc():
    nc = bacc.Bacc()
    # host-built per-image tile: [P(66x66, zero borders) ; U(P shifted
    # one row up, row 65 = 0)], bf16
    xp = nc.declare_dram_parameter("x", [NB, 128, 66, 66], bf16, isOutput=False)
    # host-packed weights: [128, 6, 128] bf16
    #   w[:, b, :]   = [W[:,0,b,:]; W[:,1,b,:]]  (tap pair a=0,1)
    #   w[:, 3+b, :] = [0         ; W[:,2,b,:]]  (tap 2, zero-padded K=128)
    w = nc.declare_dram_parameter("w", [128, 6, 128], bf16, isOutput=False)
    bias = nc.declare_dram_parameter("b", [128, 1], f32, isOutput=False)
    out = nc.declare_dram_parameter("out", [NB, 128, 64, 64], f32, isOutput=True)

    with tile.TileContext(nc) as tc, ExitStack() as ctx:
        const = ctx.enter_context(tc.tile_pool(name="const", bufs=1))
        xim_pool = ctx.enter_context(tc.tile_pool(name="xim", bufs=3))
        ob_pool = ctx.enter_context(tc.tile_pool(name="ob", bufs=2))
        ps_pool = ctx.enter_context(tc.tile_pool(name="ps", bufs=7, space="PSUM"))
        pw_pool = ctx.enter_context(tc.tile_pool(name="pw", bufs=1, space="PSUM"))

        wt = const.tile([128, 6, 128], bf16)
        bt = const.tile([128, 1], f32)
        dummy = const.tile([128, 640], bf16)

        xims = []
        for n in range(NB):
            xim = xim_pool.tile([128, 66, 66], bf16)
            if n == 0:
                # priority prefix: rows 0:20 cover blocks 0-1, then the
                # weights, so real matmuls start ~4us before the full
                # first image lands
                nc.sync.dma_start(out=xim[:, 0:20, :], in_=xp[n, :, 0:20, :])
                nc.sync.dma_start(out=wt[:], in_=w[:])
                nc.sync.dma_start(out=bt[:], in_=bias[:])
                nc.sync.dma_start(out=xim[:, 20:66, :], in_=xp[n, :, 20:66, :])
            else:
                nc.sync.dma_start(out=xim[:, :, :], in_=xp[n])
            xims.append(xim)

        # PE warm-up: independent matmuls into a scratch PSUM bank while
        # the input DMAs stream.  ~3.4us of sustained PE activity flips
        # the HAM clock gate to 2.4 GHz before the real stream begins.
        nc.gpsimd.memset(dummy[:], 0.0)
        Pw = pw_pool.tile([128, 8, 64], f32)
        for i in range(N_WARM):
            nc.tensor.matmul(
                Pw[:, :, :], dummy[:, 0:128],
                dummy[:, 128:640].rearrange("p (a b) -> p a b", a=8),
                start=True, stop=True,
            )

        for n in range(NB):
            xim = xims[n]
            osb = ob_pool.tile([128, 64, 64], f32)
            for blk in range(8):
                y0 = blk * 8
                P = ps_pool.tile([128, 8, 64], f32)
                for k, b in enumerate((0, 1, 2)):
                    nc.tensor.matmul(
                        P[:, :, :],
                        wt[:, b, :],
                        xim[:, y0:y0 + 8, b:b + 64],
                        start=(k == 0), stop=False,
                    )
                for k, b in enumerate((0, 1, 2)):
                    nc.tensor.matmul(
                        P[:, :, :],
                        wt[:, 3 + b, :],
                        xim[:, y0 + 1:y0 + 9, b:b + 64],
                        start=False, stop=(k == 2),
                    )
                # PSUM evict + bias, alternating engines per bank
                if blk % 2 == 0:
                    nc.scalar.activation(
                        osb[:, y0:y0 + 8, :], P[:, :, :],
                        mybir.ActivationFunctionType.Identity,
                        bias=bt[:, :],
                    )
                else:
                    nc.vector.tensor_scalar_add(
                        osb[:, y0:y0 + 8, :], P[:, :, :], bt[:, :])
                # drain the output in 16-row chunks so the final store
                # after the last matmul is tiny; issue from ScalarE's DGE
                if blk % 2 == 1:
                    r0 = (blk - 1) * 8
                    nc.sync.dma_start(out=out[n, :, r0:r0 + 16, :],
                                      in_=osb[:, r0:r0 + 16, :])

    nc.finalize()
    return nc


_NC = None


def _get_nc():
    global _NC
    if _NC is None:
        _NC = build_nc()
    return _NC


def _prep(inputs):
    x = np.asarray(inputs["input"], dtype=np.float32)
    w = np.asarray(inputs["weight"], dtype=np.float32)
    b = np.asarray(inputs["bias"], dtype=np.float32).reshape(128, 1)

    nimg = x.shape[0]
    xb = x.astype(ml_dtypes.bfloat16)
    xf = np.zeros((nimg, 128, 66, 66), dtype=ml_dtypes.bfloat16)
    xf[:, 0:64, 1:65, 1:65] = xb          # P: rows 1..64 = img rows 0..63
    xf[:, 64:128, 0:64, 1:65] = xb        # U: rows 0..63 = img rows 0..63

    wr = w.reshape(64, 3, 3, 128)  # [ci, a, b, co]
    wa = np.zeros((128, 6, 128), dtype=ml_dtypes.bfloat16)
    for bb in range(3):
        wa[0:64, bb, :] = wr[:, 0, bb, :]
        wa[64:128, bb, :] = wr[:, 1, bb, :]
        wa[64:128, 3 + bb, :] = wr[:, 2, bb, :]
    return xf, wa, np.ascontiguousarray(b)


def kernel(**inputs) -> np.ndarray:
    xf, wa, b = _prep(inputs)
    nc = _get_nc()
    in_maps = [
        {"x": xf[c * NB:(c + 1) * NB], "w": wa, "b": b} for c in range(N_CORES)
    ]
    res = run_bass_kernel_spmd(nc, in_maps, list(range(N_CORES)))
    return np.concatenate([r["out"] for r in res.results], axis=0)
